# revision 19
# baseline (speedup 1.0000x reference)
"""BondFastAttention Trainium2 kernel (self-contained).

Shapes (hardcoded from the problem spec):
  edge_attr [65536, 512] fp32, B=64 graphs x L=1024 bonds, HID=512, 8 heads x D=64.
  8 NeuronCores, data-parallel over graphs: G=8 graphs per core.

Device layout: transposed domain - features on partitions, tokens on free dim.
Q/K/V projections run as 3-term error-compensated fp8e4 DoubleRow matmuls
(X,W split host-side into hi+lo fp8; the lo*lo term is dropped), giving
~2.25x fewer PE cycles than bf16 at ~0.1% error. The softmax-normalizer
broadcast (rbc) is one fp8 DoubleRow matmul per chunk using mean-centered
Delta-r against exact power-of-two selector weights. All activations are
computed in a x128-scaled basis (X*4, W*32); the final LayerNorm is
scale-invariant so the scaling washes out.
"""
import numpy as np

HID = 512
HEADS = 8
D = 64
B = 64
L = 1024
SCALE = D ** -0.5
EPS = 1e-5
NCORES = 8
G = B // NCORES          # graphs per core
NCH = HID // 128         # 4 feature chunks (2 heads each)
NT = L // 128            # 8 token chunks

XS = 4.0                 # X scale
WS = 32.0                # W scale  -> q' = 128 q
PS = XS * WS             # 128: projection output scale
R0 = 1.0 / 64
DRS = 2048.0             # Delta-r quantization scale (2^11)


def _build(apply_bo: bool, apply_affine: bool):
    import concourse.bass as bass
    from concourse import bacc
    import concourse.mybir as mybir
    from concourse.tile import TileContext

    F32 = mybir.dt.float32
    F8 = mybir.dt.float8e4
    BF16 = mybir.dt.bfloat16
    AT = mybir.ActivationFunctionType
    OP = mybir.AluOpType
    DR = mybir.MatmulPerfMode.DoubleRow

    nc = bacc.Bacc()

    # Restrict activation tables so Exp/Ln/Copy/Relu/Identity resolve to one
    # physical table load (natural_log_exp_and_others).
    import concourse.bacc as _bacc_mod
    _orig_gat = _bacc_mod.get_activation_tables

    def _gat(arch):
        t = _orig_gat(arch)
        ours = {AT.Exp, AT.Ln, AT.Copy, AT.Relu, AT.Identity}
        out = {}
        for k, funcs in t.items():
            if k == "natural_log_exp_and_others":
                out[k] = funcs
            else:
                out[k] = {f for f in funcs if f not in ours}
        return out

    xhi_d = nc.dram_tensor("xhi", [HID, G * L], F8, kind="ExternalInput")
    xlo_d = nc.dram_tensor("xlo", [HID, G * L], F8, kind="ExternalInput")
    # W hi/lo per projection: [128, c(2), k(2), 512] = W'[o, 256c+128k+p]
    w_d = {}
    for name in ("q", "k", "v"):
        w_d[name + "hi"] = nc.dram_tensor(f"w{name}hi", [128, 2 * 2 * HID], F8,
                                          kind="ExternalInput")
        w_d[name + "lo"] = nc.dram_tensor(f"w{name}lo", [128, 2 * 2 * HID], F8,
                                          kind="ExternalInput")
    wot_d = nc.dram_tensor("wot", [HID, HID], BF16, kind="ExternalInput")
    wrs_d = nc.dram_tensor("wrs", [128, 128], BF16, kind="ExternalInput")
    ident_d = nc.dram_tensor("ident", [128, 128], BF16, kind="ExternalInput")
    segs_d = nc.dram_tensor("segs", [128, 8 * NCH], BF16, kind="ExternalInput")
    # rbc-DR stationary: [8, k(2), 512]: k=0 sels*2^-5, k=1 sels
    rstat_d = nc.dram_tensor("rstat", [8, 2 * HID], F8, kind="ExternalInput")
    wa_d = nc.dram_tensor("wa", [128, 1], F32, kind="ExternalInput")
    wbs_d = nc.dram_tensor("wbs", [128, 1], F32, kind="ExternalInput")
    if apply_bo:
        bod = nc.dram_tensor("bo", [1, HID], F32, kind="ExternalInput")
        onesd = nc.dram_tensor("ones1", [1, 128], F32, kind="ExternalInput")
    if apply_affine:
        lngd = nc.dram_tensor("ln_g", [128, HID], F32, kind="ExternalInput")
        lnbd = nc.dram_tensor("ln_b", [128, HID], F32, kind="ExternalInput")
    outd = nc.dram_tensor("out", [G * L, HID], F32, kind="ExternalOutput")

    with TileContext(nc) as tc:
        with tc.tile_pool(name="consts", bufs=1) as cp, \
             tc.tile_pool(name="big", bufs=1) as bp, \
             tc.tile_pool(name="small", bufs=2) as sp, \
             tc.tile_pool(name="pq", bufs=3, space="PSUM") as pqp, \
             tc.tile_pool(name="ps", bufs=3, space="PSUM") as psp, \
             tc.tile_pool(name="pr", bufs=2, space="PSUM") as prp:

            def load_x(g):
                x_sb = {}
                for nm, dt_ in (("xhi", xhi_d), ("xlo", xlo_d)):
                    xt_all = bp.tile([128, NCH * L], F8, name=f"{nm}{g}", tag=nm,
                                     bufs=2)
                    src = bass.AP(
                        tensor=dt_.ap().tensor, offset=g * L,
                        ap=[[G * L, 128], [128 * G * L, NCH], [1, L]])
                    nc.sync.dma_start(
                        out=xt_all.rearrange("p (i l) -> p i l", i=NCH), in_=src)
                    x_sb[nm] = xt_all.rearrange("p (i l) -> p i l", i=NCH)
                return x_sb

            # ---- constants to SBUF (wq + x0 first so projQ(0) starts early) ----
            w_sb = {}
            for name in ("q",):
                for hl in ("hi", "lo"):
                    t = cp.tile([128, 2 * 2 * HID], F8, name=f"w{name}{hl}")
                    nc.sync.dma_start(out=t, in_=w_d[name + hl].ap())
                    w_sb[name + hl] = t.rearrange("p (c k o) -> p c k o", c=2, k=2)
            x_states = {0: load_x(0)}
            for name in ("k", "v"):
                for hl in ("hi", "lo"):
                    t = cp.tile([128, 2 * 2 * HID], F8, name=f"w{name}{hl}")
                    nc.sync.dma_start(out=t, in_=w_d[name + hl].ap())
                    w_sb[name + hl] = t.rearrange("p (c k o) -> p c k o", c=2, k=2)
            wo_sb = [cp.tile([128, HID], BF16, name=f"wo{i}") for i in range(NCH)]
            for i in range(NCH):
                nc.sync.dma_start(out=wo_sb[i], in_=wot_d.ap()[128 * i:128 * (i + 1), :])
            wrs_sb = cp.tile([128, 128], BF16, name="wrs")
            nc.sync.dma_start(out=wrs_sb, in_=wrs_d.ap())
            ident_sb = cp.tile([128, 128], BF16, name="ident")
            nc.sync.dma_start(out=ident_sb, in_=ident_d.ap())
            segs_sb = cp.tile([128, 8 * NCH], BF16, name="segs")
            nc.sync.dma_start(out=segs_sb, in_=segs_d.ap())
            rstat_sb_ = cp.tile([8, 2 * HID], F8, name="rstat")
            nc.sync.dma_start(out=rstat_sb_, in_=rstat_d.ap())
            rstat_sb = rstat_sb_.rearrange("p (k o) -> p k o", k=2)
            wa_sb = cp.tile([128, 1], F32, name="wa")
            nc.sync.dma_start(out=wa_sb, in_=wa_d.ap())
            wbs_sb = cp.tile([128, 1], F32, name="wbs")
            nc.sync.dma_start(out=wbs_sb, in_=wbs_d.ap())
            # rbc-DR moving tiles, double-buffered by graph parity:
            # [8, k(2), 1024]: k=0 Delta-r8 (written per graph), k=1 ones.
            rmov = {}
            for st in ("A", "B"):
                for par in (0, 1):
                    t = cp.tile([8, 2 * L], F8, name=f"rmov{st}{par}")
                    nc.vector.memset(t[:, L:2 * L], 1.0)
                    rmov[(st, par)] = t
            if apply_bo:
                ones1_sb = cp.tile([1, 128], F32, name="ones1")
                nc.sync.dma_start(out=ones1_sb, in_=onesd.ap().bitcast(F32))
                bo_sb = cp.tile([1, HID], F32, name="bo")
                nc.sync.dma_start(out=bo_sb, in_=bod.ap())
            if apply_affine:
                lng_sb = cp.tile([128, HID], F32, name="lng")
                nc.sync.dma_start(out=lng_sb, in_=lngd.ap())
                lnb_sb = cp.tile([128, HID], F32, name="lnb")
                nc.sync.dma_start(out=lnb_sb, in_=lnbd.ap())

            def proj3h(name, j, n0, xhi, xlo, pp):
                """Compensated fp8 DR proj: out-chunk j, token half n0.
                K uses the plain hi@hi term only (its error is attenuated
                through the near-cancelling beta path); Q/V use 3 terms."""
                whi = w_sb[name + "hi"]
                wlo = w_sb[name + "lo"] if name != "k" else None
                o0, o1 = 128 * j, 128 * (j + 1)
                first = True
                for c in (0, 1):
                    mhi = xhi[:, 2 * c:2 * c + 2, n0:n0 + 512]
                    mlo = xlo[:, 2 * c:2 * c + 2, n0:n0 + 512]
                    whis = whi[:, c, :, o0:o1]
                    if wlo is None:
                        terms = ((whis, mhi),)
                    else:
                        wlos = wlo[:, c, :, o0:o1]
                        terms = ((whis, mhi), (whis, mlo), (wlos, mhi))
                    for ti, (wsl, msl) in enumerate(terms):
                        nc.tensor.matmul(
                            pp, wsl, msl, start=first,
                            stop=(c == 1 and ti == len(terms) - 1),
                            perf_mode=DR)
                        first = False

            def wo_ln_setup(g):
                return {
                    "mv": sp.tile([128, 2 * NT], F32, name=f"mv{g}", tag="mv"),
                    "vf": sp.tile([128, NT], F32, name=f"vf{g}", tag="vf"),
                    "lnv": sp.tile([128, NT], F32, name=f"lnv{g}", tag="lnv"),
                    "rstd": sp.tile([128, NT], F32, name=f"rstd{g}", tag="rstd"),
                    "nmr": sp.tile([128, NT], F32, name=f"nmr{g}", tag="nmr"),
                }

            def emit_wo_t(g, att_all, ws, t):
                """Wo GEMM + LayerNorm for one token chunk of graph g.
                PSUM frees via a single Act copy; LN chain runs on SBUF."""
                mv_all = ws["mv"]
                o_ps = psp.tile([8, 512], F32, name=f"ops{g}{t}", tag="seg",
                                padded_shape=[128, 512]) if False else \
                    psp.tile([128, 512], F32, name=f"ops{g}{t}", tag="seg")
                for j in range(NCH):
                    nc.tensor.matmul(
                        o_ps, att_all[:, j * L + 128 * t:j * L + 128 * (t + 1)],
                        wo_sb[j], start=(j == 0),
                        stop=(j == NCH - 1 and not apply_bo))
                if apply_bo:
                    nc.tensor.matmul(o_ps, ones1_sb, bo_sb, start=False,
                                     stop=True)
                osb = sp.tile([128, HID], F32, name=f"osb{g}{t}", tag="osb",
                              bufs=4)
                nc.scalar.copy(out=osb, in_=o_ps)
                stats = sp.tile([128, 6], F32, name=f"st{g}{t}", tag="st",
                                bufs=3)
                nc.vector.bn_stats(out=stats, in_=osb)
                nc.vector.bn_aggr(out=mv_all[:, 2 * t:2 * t + 2], in_=stats)
                nc.scalar.activation(out=ws["vf"][:, t:t + 1],
                                     in_=mv_all[:, 2 * t + 1:2 * t + 2],
                                     func=AT.Copy, bias=EPS)
                nc.scalar.activation(out=ws["lnv"][:, t:t + 1],
                                     in_=ws["vf"][:, t:t + 1], func=AT.Ln)
                nc.scalar.activation(out=ws["rstd"][:, t:t + 1],
                                     in_=ws["lnv"][:, t:t + 1],
                                     func=AT.Exp, scale=-0.5)
                # out = (osb - mu) * rstd, in place on SBUF
                nc.vector.tensor_scalar(
                    out=osb, in0=osb, scalar1=mv_all[:, 2 * t:2 * t + 1],
                    scalar2=ws["rstd"][:, t:t + 1],
                    op0=OP.subtract, op1=OP.mult)
                if apply_affine:
                    nc.vector.tensor_mul(out=osb, in0=osb, in1=lng_sb)
                    nc.vector.tensor_add(out=osb, in0=osb, in1=lnb_sb)
                nc.gpsimd.dma_start(
                    out=outd.ap()[g * L + 128 * t:g * L + 128 * (t + 1), :],
                    in_=osb)

            prev_att = None
            for g in range(G):
                par = g % 2
                x_sb = x_states.pop(g)
                if g + 1 < G:
                    x_states[g + 1] = load_x(g + 1)   # full-iteration prefetch

                # ---- Q projections: PE proj, Act e, DVE qsb copy, Pool m;
                #      segA_j trails one chunk behind to avoid PE stalls ----
                qsb_all = bp.tile([128, NCH * L], BF16, name=f"qsb{g}", tag="qsb",
                                  bufs=2)
                e_all = bp.tile([128, NCH * L], BF16, name=f"e{g}", tag="e", bufs=2)
                ksb_all = bp.tile([128, NCH * L], BF16, name=f"ksb{g}", tag="ksb",
                                  bufs=2)
                sA_lo = psp.tile([8, 512], F32, name=f"sAlo{g}", tag="seg",
                                 padded_shape=[128, 512])
                sA_hi = psp.tile([8, 512], F32, name=f"sAhi{g}", tag="seg",
                                 padded_shape=[128, 512])
                m_all = []

                def seg_emit(j, src_all, lo, hi_):
                    ech = src_all[:, j * L:(j + 1) * L]
                    for n0, s_ps in ((0, lo), (512, hi_)):
                        nc.tensor.matmul(
                            s_ps, segs_sb[:, 8 * j:8 * (j + 1)],
                            ech[:, n0:n0 + 512],
                            start=(j == 0), stop=(j == NCH - 1))

                for j in range(NCH):
                    for n0 in (0, 512):
                        pp = pqp.tile([128, 512], F32, name=f"ppq{g}{j}{n0}",
                                      tag="pq")
                        proj3h("q", j, n0, x_sb["xhi"], x_sb["xlo"], pp)
                        nc.scalar.activation(
                            out=e_all[:, j * L + n0:j * L + n0 + 512], in_=pp,
                            func=AT.Exp, scale=wa_sb)
                        nc.vector.tensor_copy(
                            out=qsb_all[:, j * L + n0:j * L + n0 + 512], in_=pp)
                    ech = e_all[:, j * L:(j + 1) * L]
                    mj = sp.tile([128, L], BF16, name=f"m{g}{j}", tag="m", bufs=5)
                    nc.gpsimd.tensor_tensor(
                        out=mj, in0=ech, in1=qsb_all[:, j * L:(j + 1) * L],
                        op=OP.mult)
                    m_all.append(mj)
                    if j >= 1:
                        seg_emit(j - 1, e_all, sA_lo, sA_hi)

                # ---- K projections (copy only; eb later from SBUF) ----
                for j in range(NCH):
                    for n0 in (0, 512):
                        pp = pqp.tile([128, 512], F32, name=f"ppk{g}{j}{n0}",
                                      tag="pq")
                        proj3h("k", j, n0, x_sb["xhi"], x_sb["xlo"], pp)
                        nc.scalar.copy(
                            out=ksb_all[:, j * L + n0:j * L + n0 + 512], in_=pp)
                    if j == 0:
                        seg_emit(NCH - 1, e_all, sA_lo, sA_hi)

                # ---- recip A + Delta-r ----
                rmovA = rmov[("A", par)]
                rtA = sp.tile([8, L], F32, name=f"rtA{g}", tag="rt")
                nc.vector.reciprocal_approx_fast(out=rtA[:, 0:512], in_=sA_lo)
                nc.vector.reciprocal_approx_fast(out=rtA[:, 512:L], in_=sA_hi)
                nc.vector.tensor_scalar(
                    out=rmovA[:, 0:L], in0=rtA, scalar1=R0, scalar2=DRS,
                    op0=OP.subtract, op1=OP.mult)

                # ---- per-j: rbcA + STT-A -> gq_j -> gqwb_j -> eb_j -> m_b_j
                #      -> segB_j, with projV_j as PE filler ----
                vsb_all = bp.tile([128, NCH * L], BF16, name=f"vsb{g}", tag="vsb",
                                  bufs=2)
                eb_all = bp.tile([128, NCH * L], BF16, name=f"eb{g}", tag="e",
                                 bufs=2)
                sB_lo = psp.tile([8, 512], F32, name=f"sBlo{g}", tag="seg",
                                 padded_shape=[128, 512])
                sB_hi = psp.tile([8, 512], F32, name=f"sBhi{g}", tag="seg",
                                 padded_shape=[128, 512])
                gqh = sp.tile([128, 2 * NCH], F32, name=f"gqh{g}", tag="gqh")
                gq = sp.tile([128, NCH], F32, name=f"gq{g}", tag="gq")
                gqwb = sp.tile([128, NCH], F32, name=f"gqwb{g}", tag="gqwb")
                scr = sp.tile([128, 512], BF16, name=f"scrA{g}", tag="scr", bufs=3)
                mb_all = []
                for j in range(NCH):
                    for hi, n0 in enumerate((0, 512)):
                        rbc = prp.tile([128, 512], F32, name=f"rbA{g}{j}{n0}",
                                       tag="pr")
                        nc.tensor.matmul(
                            rbc, rstat_sb[:, :, 128 * j:128 * (j + 1)],
                            rmovA.rearrange("p (k l) -> p k l", k=2)[:, :, n0:n0 + 512],
                            perf_mode=DR)
                        nc.vector.scalar_tensor_tensor(
                            out=scr, in0=m_all[j][:, n0:n0 + 512], scalar=R0,
                            in1=rbc, op0=OP.mult, op1=OP.mult,
                            accum_out=gqh[:, 2 * j + hi:2 * j + hi + 1])
                    nc.vector.tensor_add(out=gq[:, j:j + 1],
                                         in0=gqh[:, 2 * j:2 * j + 1],
                                         in1=gqh[:, 2 * j + 1:2 * j + 2])
                    nc.vector.tensor_scalar_mul(out=gqwb[:, j:j + 1],
                                                in0=gq[:, j:j + 1], scalar1=wbs_sb)
                    ebch = eb_all[:, j * L:(j + 1) * L]
                    nc.scalar.activation(out=ebch,
                                         in_=ksb_all[:, j * L:(j + 1) * L],
                                         func=AT.Exp, scale=gqwb[:, j:j + 1])
                    mbj = sp.tile([128, L], BF16, name=f"mb{g}{j}", tag="m", bufs=5)
                    nc.gpsimd.tensor_tensor(out=mbj, in0=ebch,
                                            in1=ksb_all[:, j * L:(j + 1) * L],
                                            op=OP.mult)
                    mb_all.append(mbj)
                    # PE filler while the eb chain runs
                    for n0 in (0, 512):
                        pp = pqp.tile([128, 512], F32, name=f"ppv{g}{j}{n0}",
                                      tag="pq")
                        proj3h("v", j, n0, x_sb["xhi"], x_sb["xlo"], pp)
                        nc.scalar.copy(
                            out=vsb_all[:, j * L + n0:j * L + n0 + 512], in_=pp)
                    seg_emit(j, eb_all, sB_lo, sB_hi)

                # ---- recip B + Delta-r ----
                rmovB = rmov[("B", par)]
                rtB = sp.tile([8, L], F32, name=f"rtB{g}", tag="rt")
                nc.vector.reciprocal_approx_fast(out=rtB[:, 0:512], in_=sB_lo)
                nc.vector.reciprocal_approx_fast(out=rtB[:, 512:L], in_=sB_hi)
                nc.vector.tensor_scalar(
                    out=rmovB[:, 0:L], in0=rtB, scalar1=R0, scalar2=DRS,
                    op0=OP.subtract, op1=OP.mult)

                # ---- per-j: rbcB + STT-B -> gk_j -> kv_j (DVE 4x) -> kvout_j
                #      -> att_j, with Wo/LN(g-1) token-pairs as PE filler ----
                att_all = bp.tile([128, NCH * L], BF16, name=f"att{g}", tag="att",
                                  bufs=2)
                acch = sp.tile([128, 2 * NCH], F32, name=f"acch{g}", tag="gqh")
                gk = sp.tile([128, NCH], F32, name=f"gk{g}", tag="gk")
                scrb = sp.tile([128, 512], BF16, name=f"scrB{g}", tag="scr", bufs=3)
                if prev_att is not None:
                    ws_prev = wo_ln_setup(g - 1)
                for j in range(NCH):
                    for hi, n0 in enumerate((0, 512)):
                        rbc = prp.tile([128, 512], F32, name=f"rbB{g}{j}{n0}",
                                       tag="pr")
                        nc.tensor.matmul(
                            rbc, rstat_sb[:, :, 128 * j:128 * (j + 1)],
                            rmovB.rearrange("p (k l) -> p k l", k=2)[:, :, n0:n0 + 512],
                            perf_mode=DR)
                        nc.vector.scalar_tensor_tensor(
                            out=scrb, in0=mb_all[j][:, n0:n0 + 512], scalar=R0,
                            in1=rbc, op0=OP.mult, op1=OP.mult,
                            accum_out=acch[:, 2 * j + hi:2 * j + hi + 1])
                    nc.vector.tensor_add(out=acch[:, 2 * j:2 * j + 1],
                                         in0=acch[:, 2 * j:2 * j + 1],
                                         in1=acch[:, 2 * j + 1:2 * j + 2])
                    nc.vector.tensor_mul(out=gk[:, j:j + 1],
                                         in0=acch[:, 2 * j:2 * j + 1],
                                         in1=gq[:, j:j + 1])
                    kvj = sp.tile([128, L], BF16, name=f"kv{g}{j}", tag="kv",
                                  bufs=3)
                    nc.vector.tensor_scalar_mul(out=kvj,
                                                in0=vsb_all[:, j * L:(j + 1) * L],
                                                scalar1=gk[:, j:j + 1])
                    for n0 in (0, 512):
                        kvo = prp.tile([128, 512], F32, name=f"kvo{g}{j}{n0}",
                                       tag="pr")
                        nc.tensor.matmul(kvo, ident_sb,
                                         qsb_all[:, j * L + n0:j * L + n0 + 512],
                                         start=True, stop=False)
                        nc.tensor.matmul(kvo, wrs_sb, kvj[:, n0:n0 + 512],
                                         start=False, stop=True)
                        nc.scalar.activation(
                            out=att_all[:, j * L + n0:j * L + n0 + 512],
                            in_=kvo, func=AT.Relu)
                    if prev_att is not None:
                        emit_wo_t(g - 1, prev_att, ws_prev, 2 * j)
                        emit_wo_t(g - 1, prev_att, ws_prev, 2 * j + 1)
                prev_att = att_all

            ws_last = wo_ln_setup(G - 1)
            for t in range(NT):
                emit_wo_t(G - 1, prev_att, ws_last, t)

    _bacc_mod.get_activation_tables = _gat
    try:
        nc.compile()
    finally:
        _bacc_mod.get_activation_tables = _orig_gat
    return nc


_NC_CACHE = {}


def _get_nc(apply_bo=False, apply_affine=False):
    key = (apply_bo, apply_affine)
    if key not in _NC_CACHE:
        _NC_CACHE[key] = _build(apply_bo, apply_affine)
    return _NC_CACHE[key]


def _host_consts(Wq, Wk, Wv, Wr, w_alpha, w_beta, Wo, bo, ln_g, ln_b):
    import ml_dtypes
    f8 = ml_dtypes.float8_e4m3
    bf = ml_dtypes.bfloat16

    def whl(W):
        # [128, c, k, o] with value W'[o, 256c+128k+p] * WS, split hi/lo fp8
        Ws = (W.T * WS).astype(np.float32)          # [h, o]
        t = Ws.reshape(2, 2, 128, HID).transpose(2, 0, 1, 3)   # [p, c, k, o]
        hi = t.astype(f8)
        lo = (t - hi.astype(np.float32)).astype(f8)
        return (np.ascontiguousarray(hi.reshape(128, 2 * 2 * HID)),
                np.ascontiguousarray(lo.reshape(128, 2 * 2 * HID)))

    qhi, qlo = whl(Wq)
    khi, klo = whl(Wk)
    vhi, vlo = whl(Wv)
    wot = np.ascontiguousarray(Wo.T).astype(bf)
    # wrs = block-diag(Wr.T) * PS / (PS^2 * gk-scale) ... in the scaled basis:
    # kv' = 2^21 kv_true, want kvout' = PS*kvout_true -> wrs = wrbd * 2^-14
    wrt = Wr.T.astype(np.float32)
    wrbd = np.zeros((128, 128), np.float32)
    wrbd[:64, :64] = wrt
    wrbd[64:, 64:] = wrt
    wrs = (wrbd * (1.0 / (PS * PS))).astype(bf)
    ident = np.eye(128, dtype=np.float32).astype(bf)
    segs = np.zeros((128, 8 * NCH), np.float32)
    for j in range(NCH):
        for p in range(128):
            segs[p, 8 * j + 2 * j + p // 64] = 1.0
    # rstat [8, k(2), 512]: k=0 sels * 2^(6-11) = 2^-5, k=1 sels (exact 1.0)
    sels = np.zeros((8, HID), np.float32)
    for m in range(HID):
        sels[2 * (m // 128) + (m % 128) // 64, m] = 1.0
    rstat = np.stack([sels * (64.0 / DRS), sels], axis=1).reshape(8, 2 * HID)
    wa_col = (np.tile(w_alpha, 2) * SCALE / PS).reshape(128, 1).astype(np.float32)
    wbs_col = (np.tile(w_beta, 2) * SCALE / (PS * PS)).reshape(128, 1).astype(
        np.float32)

    common = {"wqhi": qhi, "wqlo": qlo, "wkhi": khi, "wklo": klo,
              "wvhi": vhi, "wvlo": vlo, "wot": wot, "wrs": wrs,
              "ident": ident, "segs": segs.astype(bf),
              "rstat": rstat.astype(f8), "wa": wa_col, "wbs": wbs_col}
    apply_bo = not np.allclose(bo, 0.0)
    apply_affine = not (np.allclose(ln_g, 1.0) and np.allclose(ln_b, 0.0))
    if apply_bo:
        common["bo"] = (bo * PS).reshape(1, HID).astype(np.float32)
        common["ones1"] = np.ones((1, 128), np.float32)
    if apply_affine:
        common["ln_g"] = np.tile(ln_g, (128, 1)).astype(np.float32)
        common["ln_b"] = np.tile(ln_b, (128, 1)).astype(np.float32)
    return common, apply_bo, apply_affine


def kernel(edge_attr, batch_scopes, Wq, Wk, Wv, Wr, w_alpha, w_beta, Wo, bo,
           ln_g, ln_b):
    from concourse import bass_utils
    import ml_dtypes
    f8 = ml_dtypes.float8_e4m3

    edge_attr = np.asarray(edge_attr, dtype=np.float32)
    scopes = np.asarray(batch_scopes)
    Wq = np.asarray(Wq, np.float32); Wk = np.asarray(Wk, np.float32)
    Wv = np.asarray(Wv, np.float32); Wr = np.asarray(Wr, np.float32)
    Wo = np.asarray(Wo, np.float32)
    w_alpha = np.asarray(w_alpha, np.float32)
    w_beta = np.asarray(w_beta, np.float32)
    bo = np.asarray(bo, np.float32)
    ln_g = np.asarray(ln_g, np.float32); ln_b = np.asarray(ln_b, np.float32)

    assert np.all(scopes[:, 1] == L), "equal-length contiguous scopes expected"
    starts = scopes[:, 0].astype(np.int64)

    common, apply_bo, apply_affine = _host_consts(
        Wq, Wk, Wv, Wr, w_alpha, w_beta, Wo, bo, ln_g, ln_b)
    nc = _get_nc(apply_bo, apply_affine)

    in_maps = []
    for c in range(NCORES):
        rows = np.concatenate([
            np.arange(starts[c * G + g], starts[c * G + g] + L)
            for g in range(G)])
        xs = np.ascontiguousarray(edge_attr[rows].T) * XS     # [512, G*L]
        xhi = xs.astype(f8)
        xlo = (xs - xhi.astype(np.float32)).astype(f8)
        in_maps.append({"xhi": xhi, "xlo": xlo, **common})

    res = bass_utils.run_bass_kernel_spmd(nc, in_maps, core_ids=list(range(NCORES)))
    out = np.concatenate([r["out"] for r in res.results], axis=0)
    return out.astype(np.float32)


# revision 20
# speedup vs baseline: 1.0142x; 1.0142x over previous
"""BondFastAttention Trainium2 kernel (self-contained).

Shapes (hardcoded from the problem spec):
  edge_attr [65536, 512] fp32, B=64 graphs x L=1024 bonds, HID=512, 8 heads x D=64.
  8 NeuronCores, data-parallel over graphs: G=8 graphs per core.

Device layout: transposed domain - features on partitions, tokens on free dim.
Q/K/V projections run as 3-term error-compensated fp8e4 DoubleRow matmuls
(X,W split host-side into hi+lo fp8; the lo*lo term is dropped), giving
~2.25x fewer PE cycles than bf16 at ~0.1% error. The softmax-normalizer
broadcast (rbc) is one fp8 DoubleRow matmul per chunk using mean-centered
Delta-r against exact power-of-two selector weights. All activations are
computed in a x128-scaled basis (X*4, W*32); the final LayerNorm is
scale-invariant so the scaling washes out.
"""
import numpy as np

HID = 512
HEADS = 8
D = 64
B = 64
L = 1024
SCALE = D ** -0.5
EPS = 1e-5
NCORES = 8
G = B // NCORES          # graphs per core
NCH = HID // 128         # 4 feature chunks (2 heads each)
NT = L // 128            # 8 token chunks

XS = 4.0                 # X scale
WS = 32.0                # W scale  -> q' = 128 q
PS = XS * WS             # 128: projection output scale
R0 = 1.0 / 64
DRS = 2048.0             # Delta-r quantization scale (2^11)


def _build(apply_bo: bool, apply_affine: bool):
    import concourse.bass as bass
    from concourse import bacc
    import concourse.mybir as mybir
    from concourse.tile import TileContext

    F32 = mybir.dt.float32
    F8 = mybir.dt.float8e4
    BF16 = mybir.dt.bfloat16
    AT = mybir.ActivationFunctionType
    OP = mybir.AluOpType
    DR = mybir.MatmulPerfMode.DoubleRow

    nc = bacc.Bacc()

    # Restrict activation tables so Exp/Ln/Copy/Relu/Identity resolve to one
    # physical table load (natural_log_exp_and_others).
    import concourse.bacc as _bacc_mod
    _orig_gat = _bacc_mod.get_activation_tables

    def _gat(arch):
        t = _orig_gat(arch)
        ours = {AT.Exp, AT.Ln, AT.Copy, AT.Relu, AT.Identity}
        out = {}
        for k, funcs in t.items():
            if k == "natural_log_exp_and_others":
                out[k] = funcs
            else:
                out[k] = {f for f in funcs if f not in ours}
        return out

    xhi_d = nc.dram_tensor("xhi", [HID, G * L], F8, kind="ExternalInput")
    xlo_d = nc.dram_tensor("xlo", [HID, G * L], F8, kind="ExternalInput")
    # W hi/lo per projection: [128, c(2), k(2), 512] = W'[o, 256c+128k+p]
    w_d = {}
    for name in ("q", "k", "v"):
        w_d[name + "hi"] = nc.dram_tensor(f"w{name}hi", [128, 2 * 2 * HID], F8,
                                          kind="ExternalInput")
        w_d[name + "lo"] = nc.dram_tensor(f"w{name}lo", [128, 2 * 2 * HID], F8,
                                          kind="ExternalInput")
    wot_d = nc.dram_tensor("wot", [HID, HID], BF16, kind="ExternalInput")
    wrs_d = nc.dram_tensor("wrs", [128, 128], BF16, kind="ExternalInput")
    ident_d = nc.dram_tensor("ident", [128, 128], BF16, kind="ExternalInput")
    segs_d = nc.dram_tensor("segs", [128, 8 * NCH], BF16, kind="ExternalInput")
    # rbc-DR stationary: [8, k(2), 512]: k=0 sels*2^-5, k=1 sels
    rstat_d = nc.dram_tensor("rstat", [8, 2 * HID], F8, kind="ExternalInput")
    wa_d = nc.dram_tensor("wa", [128, 1], F32, kind="ExternalInput")
    wbs_d = nc.dram_tensor("wbs", [128, 1], F32, kind="ExternalInput")
    if apply_bo:
        bod = nc.dram_tensor("bo", [1, HID], F32, kind="ExternalInput")
        onesd = nc.dram_tensor("ones1", [1, 128], F32, kind="ExternalInput")
    if apply_affine:
        lngd = nc.dram_tensor("ln_g", [128, HID], F32, kind="ExternalInput")
        lnbd = nc.dram_tensor("ln_b", [128, HID], F32, kind="ExternalInput")
    outd = nc.dram_tensor("out", [G * L, HID], F32, kind="ExternalOutput")

    with TileContext(nc) as tc:
        with tc.tile_pool(name="consts", bufs=1) as cp, \
             tc.tile_pool(name="big", bufs=1) as bp, \
             tc.tile_pool(name="small", bufs=2) as sp, \
             tc.tile_pool(name="pq", bufs=3, space="PSUM") as pqp, \
             tc.tile_pool(name="ps", bufs=2, space="PSUM") as psp, \
             tc.tile_pool(name="pr", bufs=3, space="PSUM") as prp:

            def load_x(g):
                x_sb = {}
                for nm, dt_ in (("xhi", xhi_d), ("xlo", xlo_d)):
                    xt_all = bp.tile([128, NCH * L], F8, name=f"{nm}{g}", tag=nm,
                                     bufs=2)
                    src = bass.AP(
                        tensor=dt_.ap().tensor, offset=g * L,
                        ap=[[G * L, 128], [128 * G * L, NCH], [1, L]])
                    nc.sync.dma_start(
                        out=xt_all.rearrange("p (i l) -> p i l", i=NCH), in_=src)
                    x_sb[nm] = xt_all.rearrange("p (i l) -> p i l", i=NCH)
                return x_sb

            # ---- constants to SBUF (wq + x0 first so projQ(0) starts early) ----
            w_sb = {}
            for name in ("q",):
                for hl in ("hi", "lo"):
                    t = cp.tile([128, 2 * 2 * HID], F8, name=f"w{name}{hl}")
                    nc.sync.dma_start(out=t, in_=w_d[name + hl].ap())
                    w_sb[name + hl] = t.rearrange("p (c k o) -> p c k o", c=2, k=2)
            x_states = {0: load_x(0)}
            for name in ("k", "v"):
                for hl in ("hi", "lo"):
                    t = cp.tile([128, 2 * 2 * HID], F8, name=f"w{name}{hl}")
                    nc.sync.dma_start(out=t, in_=w_d[name + hl].ap())
                    w_sb[name + hl] = t.rearrange("p (c k o) -> p c k o", c=2, k=2)
            wo_sb = [cp.tile([128, HID], BF16, name=f"wo{i}") for i in range(NCH)]
            for i in range(NCH):
                nc.sync.dma_start(out=wo_sb[i], in_=wot_d.ap()[128 * i:128 * (i + 1), :])
            wrs_sb = cp.tile([128, 128], BF16, name="wrs")
            nc.sync.dma_start(out=wrs_sb, in_=wrs_d.ap())
            ident_sb = cp.tile([128, 128], BF16, name="ident")
            nc.sync.dma_start(out=ident_sb, in_=ident_d.ap())
            segs_sb = cp.tile([128, 8 * NCH], BF16, name="segs")
            nc.sync.dma_start(out=segs_sb, in_=segs_d.ap())
            rstat_sb_ = cp.tile([8, 2 * HID], F8, name="rstat")
            nc.sync.dma_start(out=rstat_sb_, in_=rstat_d.ap())
            rstat_sb = rstat_sb_.rearrange("p (k o) -> p k o", k=2)
            wa_sb = cp.tile([128, 1], F32, name="wa")
            nc.sync.dma_start(out=wa_sb, in_=wa_d.ap())
            wbs_sb = cp.tile([128, 1], F32, name="wbs")
            nc.sync.dma_start(out=wbs_sb, in_=wbs_d.ap())
            # rbc-DR moving tiles, double-buffered by graph parity:
            # [8, k(2), 1024]: k=0 Delta-r8 (written per graph), k=1 ones.
            rmov = {}
            for st in ("A", "B"):
                for par in (0, 1):
                    t = cp.tile([8, 2 * L], F8, name=f"rmov{st}{par}")
                    nc.vector.memset(t[:, L:2 * L], 1.0)
                    rmov[(st, par)] = t
            if apply_bo:
                ones1_sb = cp.tile([1, 128], F32, name="ones1")
                nc.sync.dma_start(out=ones1_sb, in_=onesd.ap().bitcast(F32))
                bo_sb = cp.tile([1, HID], F32, name="bo")
                nc.sync.dma_start(out=bo_sb, in_=bod.ap())
            if apply_affine:
                lng_sb = cp.tile([128, HID], F32, name="lng")
                nc.sync.dma_start(out=lng_sb, in_=lngd.ap())
                lnb_sb = cp.tile([128, HID], F32, name="lnb")
                nc.sync.dma_start(out=lnb_sb, in_=lnbd.ap())

            def proj3h(name, j, n0, xhi, xlo, pp):
                """Compensated fp8 DR proj: out-chunk j, token half n0.
                K uses the plain hi@hi term only (its error is attenuated
                through the near-cancelling beta path); Q/V use 3 terms."""
                whi = w_sb[name + "hi"]
                wlo = w_sb[name + "lo"] if name != "k" else None
                o0, o1 = 128 * j, 128 * (j + 1)
                first = True
                for c in (0, 1):
                    mhi = xhi[:, 2 * c:2 * c + 2, n0:n0 + 512]
                    mlo = xlo[:, 2 * c:2 * c + 2, n0:n0 + 512]
                    whis = whi[:, c, :, o0:o1]
                    if wlo is None:
                        terms = ((whis, mhi),)
                    else:
                        wlos = wlo[:, c, :, o0:o1]
                        terms = ((whis, mhi), (whis, mlo), (wlos, mhi))
                    for ti, (wsl, msl) in enumerate(terms):
                        nc.tensor.matmul(
                            pp, wsl, msl, start=first,
                            stop=(c == 1 and ti == len(terms) - 1),
                            perf_mode=DR)
                        first = False

            def wo_ln_setup(g):
                return {
                    "mv": sp.tile([128, 2 * NT], F32, name=f"mv{g}", tag="mv"),
                    "vf": sp.tile([128, NT], F32, name=f"vf{g}", tag="vf"),
                    "lnv": sp.tile([128, NT], F32, name=f"lnv{g}", tag="lnv"),
                    "rstd": sp.tile([128, NT], F32, name=f"rstd{g}", tag="rstd"),
                    "nmr": sp.tile([128, NT], F32, name=f"nmr{g}", tag="nmr"),
                }

            def emit_wo_t(g, att_all, ws, t):
                """Wo GEMM + LayerNorm for one token chunk of graph g.
                PSUM frees via a single Act copy; LN chain runs on SBUF."""
                mv_all = ws["mv"]
                o_ps = psp.tile([8, 512], F32, name=f"ops{g}{t}", tag="seg",
                                padded_shape=[128, 512]) if False else \
                    psp.tile([128, 512], F32, name=f"ops{g}{t}", tag="seg")
                for j in range(NCH):
                    nc.tensor.matmul(
                        o_ps, att_all[:, j * L + 128 * t:j * L + 128 * (t + 1)],
                        wo_sb[j], start=(j == 0),
                        stop=(j == NCH - 1 and not apply_bo))
                if apply_bo:
                    nc.tensor.matmul(o_ps, ones1_sb, bo_sb, start=False,
                                     stop=True)
                osb = sp.tile([128, HID], F32, name=f"osb{g}{t}", tag="osb",
                              bufs=4)
                nc.scalar.copy(out=osb, in_=o_ps)
                stats = sp.tile([128, 6], F32, name=f"st{g}{t}", tag="st",
                                bufs=3)
                nc.vector.bn_stats(out=stats, in_=osb)
                nc.vector.bn_aggr(out=mv_all[:, 2 * t:2 * t + 2], in_=stats)
                nc.scalar.activation(out=ws["vf"][:, t:t + 1],
                                     in_=mv_all[:, 2 * t + 1:2 * t + 2],
                                     func=AT.Copy, bias=EPS)
                nc.scalar.activation(out=ws["lnv"][:, t:t + 1],
                                     in_=ws["vf"][:, t:t + 1], func=AT.Ln)
                nc.scalar.activation(out=ws["rstd"][:, t:t + 1],
                                     in_=ws["lnv"][:, t:t + 1],
                                     func=AT.Exp, scale=-0.5)
                # out = (osb - mu) * rstd, in place on SBUF
                nc.vector.tensor_scalar(
                    out=osb, in0=osb, scalar1=mv_all[:, 2 * t:2 * t + 1],
                    scalar2=ws["rstd"][:, t:t + 1],
                    op0=OP.subtract, op1=OP.mult)
                if apply_affine:
                    nc.vector.tensor_mul(out=osb, in0=osb, in1=lng_sb)
                    nc.vector.tensor_add(out=osb, in0=osb, in1=lnb_sb)
                nc.gpsimd.dma_start(
                    out=outd.ap()[g * L + 128 * t:g * L + 128 * (t + 1), :],
                    in_=osb)

            prev_att = None
            for g in range(G):
                par = g % 2
                x_sb = x_states.pop(g)
                if g + 1 < G:
                    x_states[g + 1] = load_x(g + 1)   # full-iteration prefetch

                # ---- Q projections: PE proj, Act e, DVE qsb copy, Pool m;
                #      segA_j trails one chunk behind to avoid PE stalls ----
                qsb_all = bp.tile([128, NCH * L], BF16, name=f"qsb{g}", tag="qsb",
                                  bufs=2)
                e_all = bp.tile([128, NCH * L], BF16, name=f"e{g}", tag="e", bufs=2)
                ksb_all = bp.tile([128, NCH * L], BF16, name=f"ksb{g}", tag="ksb",
                                  bufs=2)
                sA_lo = psp.tile([8, 512], F32, name=f"sAlo{g}", tag="seg",
                                 padded_shape=[128, 512])
                sA_hi = psp.tile([8, 512], F32, name=f"sAhi{g}", tag="seg",
                                 padded_shape=[128, 512])
                m_all = []

                def seg_emit(j, src_all, lo, hi_):
                    ech = src_all[:, j * L:(j + 1) * L]
                    for n0, s_ps in ((0, lo), (512, hi_)):
                        nc.tensor.matmul(
                            s_ps, segs_sb[:, 8 * j:8 * (j + 1)],
                            ech[:, n0:n0 + 512],
                            start=(j == 0), stop=(j == NCH - 1))

                for j in range(NCH):
                    for n0 in (0, 512):
                        pp = pqp.tile([128, 512], F32, name=f"ppq{g}{j}{n0}",
                                      tag="pq")
                        proj3h("q", j, n0, x_sb["xhi"], x_sb["xlo"], pp)
                        nc.scalar.activation(
                            out=e_all[:, j * L + n0:j * L + n0 + 512], in_=pp,
                            func=AT.Exp, scale=wa_sb)
                        nc.vector.tensor_copy(
                            out=qsb_all[:, j * L + n0:j * L + n0 + 512], in_=pp)
                    ech = e_all[:, j * L:(j + 1) * L]
                    mj = sp.tile([128, L], BF16, name=f"m{g}{j}", tag="m", bufs=5)
                    nc.vector.tensor_tensor(
                        out=mj, in0=ech, in1=qsb_all[:, j * L:(j + 1) * L],
                        op=OP.mult)
                    m_all.append(mj)
                    if j >= 1:
                        seg_emit(j - 1, e_all, sA_lo, sA_hi)

                # ---- K projections (copy only; eb later from SBUF) ----
                for j in range(NCH):
                    for n0 in (0, 512):
                        pp = pqp.tile([128, 512], F32, name=f"ppk{g}{j}{n0}",
                                      tag="pq")
                        proj3h("k", j, n0, x_sb["xhi"], x_sb["xlo"], pp)
                        nc.scalar.copy(
                            out=ksb_all[:, j * L + n0:j * L + n0 + 512], in_=pp)
                    if j == 0:
                        seg_emit(NCH - 1, e_all, sA_lo, sA_hi)

                # ---- recip A + Delta-r ----
                rmovA = rmov[("A", par)]
                rtA = sp.tile([8, L], F32, name=f"rtA{g}", tag="rt")
                nc.vector.reciprocal_approx_fast(out=rtA[:, 0:512], in_=sA_lo)
                nc.vector.reciprocal_approx_fast(out=rtA[:, 512:L], in_=sA_hi)
                nc.vector.tensor_scalar(
                    out=rmovA[:, 0:L], in0=rtA, scalar1=R0, scalar2=DRS,
                    op0=OP.subtract, op1=OP.mult)

                # ---- per-j: rbcA + STT-A -> gq_j -> gqwb_j -> eb_j -> m_b_j
                #      -> segB_j, with projV_j as PE filler ----
                vsb_all = bp.tile([128, NCH * L], BF16, name=f"vsb{g}", tag="vsb",
                                  bufs=2)
                eb_all = bp.tile([128, NCH * L], BF16, name=f"eb{g}", tag="e",
                                 bufs=2)
                sB_lo = psp.tile([8, 512], F32, name=f"sBlo{g}", tag="seg",
                                 padded_shape=[128, 512])
                sB_hi = psp.tile([8, 512], F32, name=f"sBhi{g}", tag="seg",
                                 padded_shape=[128, 512])
                gqh = sp.tile([128, 2 * NCH], F32, name=f"gqh{g}", tag="gqh")
                gq = sp.tile([128, NCH], F32, name=f"gq{g}", tag="gq")
                gqwb = sp.tile([128, NCH], F32, name=f"gqwb{g}", tag="gqwb")
                scr = sp.tile([128, 512], BF16, name=f"scrA{g}", tag="scr", bufs=3)
                mb_all = []
                for j in range(NCH):
                    for hi, n0 in enumerate((0, 512)):
                        rbc = prp.tile([128, 512], F32, name=f"rbA{g}{j}{n0}",
                                       tag="pr")
                        nc.tensor.matmul(
                            rbc, rstat_sb[:, :, 128 * j:128 * (j + 1)],
                            rmovA.rearrange("p (k l) -> p k l", k=2)[:, :, n0:n0 + 512],
                            perf_mode=DR)
                        nc.vector.scalar_tensor_tensor(
                            out=scr, in0=m_all[j][:, n0:n0 + 512], scalar=R0,
                            in1=rbc, op0=OP.mult, op1=OP.mult,
                            accum_out=gqh[:, 2 * j + hi:2 * j + hi + 1])
                    nc.vector.tensor_add(out=gq[:, j:j + 1],
                                         in0=gqh[:, 2 * j:2 * j + 1],
                                         in1=gqh[:, 2 * j + 1:2 * j + 2])
                    nc.vector.tensor_scalar_mul(out=gqwb[:, j:j + 1],
                                                in0=gq[:, j:j + 1], scalar1=wbs_sb)
                    ebch = eb_all[:, j * L:(j + 1) * L]
                    nc.scalar.activation(out=ebch,
                                         in_=ksb_all[:, j * L:(j + 1) * L],
                                         func=AT.Exp, scale=gqwb[:, j:j + 1])
                    mbj = sp.tile([128, L], BF16, name=f"mb{g}{j}", tag="m", bufs=5)
                    nc.gpsimd.tensor_tensor(out=mbj, in0=ebch,
                                            in1=ksb_all[:, j * L:(j + 1) * L],
                                            op=OP.mult)
                    mb_all.append(mbj)
                    # PE filler while the eb chain runs
                    for n0 in (0, 512):
                        pp = pqp.tile([128, 512], F32, name=f"ppv{g}{j}{n0}",
                                      tag="pq")
                        proj3h("v", j, n0, x_sb["xhi"], x_sb["xlo"], pp)
                        nc.scalar.copy(
                            out=vsb_all[:, j * L + n0:j * L + n0 + 512], in_=pp)
                    seg_emit(j, eb_all, sB_lo, sB_hi)

                # ---- recip B + Delta-r ----
                rmovB = rmov[("B", par)]
                rtB = sp.tile([8, L], F32, name=f"rtB{g}", tag="rt")
                nc.vector.reciprocal_approx_fast(out=rtB[:, 0:512], in_=sB_lo)
                nc.vector.reciprocal_approx_fast(out=rtB[:, 512:L], in_=sB_hi)
                nc.vector.tensor_scalar(
                    out=rmovB[:, 0:L], in0=rtB, scalar1=R0, scalar2=DRS,
                    op0=OP.subtract, op1=OP.mult)

                # ---- per-j: rbcB + STT-B -> gk_j -> kv_j (DVE 4x) -> kvout_j
                #      -> att_j, with Wo/LN(g-1) token-pairs as PE filler ----
                att_all = bp.tile([128, NCH * L], BF16, name=f"att{g}", tag="att",
                                  bufs=2)
                acch = sp.tile([128, 2 * NCH], F32, name=f"acch{g}", tag="gqh")
                gk = sp.tile([128, NCH], F32, name=f"gk{g}", tag="gk")
                scrb = sp.tile([128, 512], BF16, name=f"scrB{g}", tag="scr", bufs=3)
                if prev_att is not None:
                    ws_prev = wo_ln_setup(g - 1)
                for j in range(NCH):
                    for hi, n0 in enumerate((0, 512)):
                        rbc = prp.tile([128, 512], F32, name=f"rbB{g}{j}{n0}",
                                       tag="pr")
                        nc.tensor.matmul(
                            rbc, rstat_sb[:, :, 128 * j:128 * (j + 1)],
                            rmovB.rearrange("p (k l) -> p k l", k=2)[:, :, n0:n0 + 512],
                            perf_mode=DR)
                        nc.vector.scalar_tensor_tensor(
                            out=scrb, in0=mb_all[j][:, n0:n0 + 512], scalar=R0,
                            in1=rbc, op0=OP.mult, op1=OP.mult,
                            accum_out=acch[:, 2 * j + hi:2 * j + hi + 1])
                    nc.vector.tensor_add(out=acch[:, 2 * j:2 * j + 1],
                                         in0=acch[:, 2 * j:2 * j + 1],
                                         in1=acch[:, 2 * j + 1:2 * j + 2])
                    nc.vector.tensor_mul(out=gk[:, j:j + 1],
                                         in0=acch[:, 2 * j:2 * j + 1],
                                         in1=gq[:, j:j + 1])
                    kvj = sp.tile([128, L], BF16, name=f"kv{g}{j}", tag="kv",
                                  bufs=3)
                    nc.vector.tensor_scalar_mul(out=kvj,
                                                in0=vsb_all[:, j * L:(j + 1) * L],
                                                scalar1=gk[:, j:j + 1])
                    for n0 in (0, 512):
                        kvo = prp.tile([128, 512], F32, name=f"kvo{g}{j}{n0}",
                                       tag="pr")
                        nc.tensor.matmul(kvo, ident_sb,
                                         qsb_all[:, j * L + n0:j * L + n0 + 512],
                                         start=True, stop=False)
                        nc.tensor.matmul(kvo, wrs_sb, kvj[:, n0:n0 + 512],
                                         start=False, stop=True)
                        nc.scalar.activation(
                            out=att_all[:, j * L + n0:j * L + n0 + 512],
                            in_=kvo, func=AT.Relu)
                    if prev_att is not None:
                        emit_wo_t(g - 1, prev_att, ws_prev, 2 * j)
                        emit_wo_t(g - 1, prev_att, ws_prev, 2 * j + 1)
                prev_att = att_all

            ws_last = wo_ln_setup(G - 1)
            for t in range(NT):
                emit_wo_t(G - 1, prev_att, ws_last, t)

    _bacc_mod.get_activation_tables = _gat
    try:
        nc.compile()
    finally:
        _bacc_mod.get_activation_tables = _orig_gat
    return nc


_NC_CACHE = {}


def _get_nc(apply_bo=False, apply_affine=False):
    key = (apply_bo, apply_affine)
    if key not in _NC_CACHE:
        _NC_CACHE[key] = _build(apply_bo, apply_affine)
    return _NC_CACHE[key]


def _host_consts(Wq, Wk, Wv, Wr, w_alpha, w_beta, Wo, bo, ln_g, ln_b):
    import ml_dtypes
    f8 = ml_dtypes.float8_e4m3
    bf = ml_dtypes.bfloat16

    def whl(W):
        # [128, c, k, o] with value W'[o, 256c+128k+p] * WS, split hi/lo fp8
        Ws = (W.T * WS).astype(np.float32)          # [h, o]
        t = Ws.reshape(2, 2, 128, HID).transpose(2, 0, 1, 3)   # [p, c, k, o]
        hi = t.astype(f8)
        lo = (t - hi.astype(np.float32)).astype(f8)
        return (np.ascontiguousarray(hi.reshape(128, 2 * 2 * HID)),
                np.ascontiguousarray(lo.reshape(128, 2 * 2 * HID)))

    qhi, qlo = whl(Wq)
    khi, klo = whl(Wk)
    vhi, vlo = whl(Wv)
    wot = np.ascontiguousarray(Wo.T).astype(bf)
    # wrs = block-diag(Wr.T) * PS / (PS^2 * gk-scale) ... in the scaled basis:
    # kv' = 2^21 kv_true, want kvout' = PS*kvout_true -> wrs = wrbd * 2^-14
    wrt = Wr.T.astype(np.float32)
    wrbd = np.zeros((128, 128), np.float32)
    wrbd[:64, :64] = wrt
    wrbd[64:, 64:] = wrt
    wrs = (wrbd * (1.0 / (PS * PS))).astype(bf)
    ident = np.eye(128, dtype=np.float32).astype(bf)
    segs = np.zeros((128, 8 * NCH), np.float32)
    for j in range(NCH):
        for p in range(128):
            segs[p, 8 * j + 2 * j + p // 64] = 1.0
    # rstat [8, k(2), 512]: k=0 sels * 2^(6-11) = 2^-5, k=1 sels (exact 1.0)
    sels = np.zeros((8, HID), np.float32)
    for m in range(HID):
        sels[2 * (m // 128) + (m % 128) // 64, m] = 1.0
    rstat = np.stack([sels * (64.0 / DRS), sels], axis=1).reshape(8, 2 * HID)
    wa_col = (np.tile(w_alpha, 2) * SCALE / PS).reshape(128, 1).astype(np.float32)
    wbs_col = (np.tile(w_beta, 2) * SCALE / (PS * PS)).reshape(128, 1).astype(
        np.float32)

    common = {"wqhi": qhi, "wqlo": qlo, "wkhi": khi, "wklo": klo,
              "wvhi": vhi, "wvlo": vlo, "wot": wot, "wrs": wrs,
              "ident": ident, "segs": segs.astype(bf),
              "rstat": rstat.astype(f8), "wa": wa_col, "wbs": wbs_col}
    apply_bo = not np.allclose(bo, 0.0)
    apply_affine = not (np.allclose(ln_g, 1.0) and np.allclose(ln_b, 0.0))
    if apply_bo:
        common["bo"] = (bo * PS).reshape(1, HID).astype(np.float32)
        common["ones1"] = np.ones((1, 128), np.float32)
    if apply_affine:
        common["ln_g"] = np.tile(ln_g, (128, 1)).astype(np.float32)
        common["ln_b"] = np.tile(ln_b, (128, 1)).astype(np.float32)
    return common, apply_bo, apply_affine


def kernel(edge_attr, batch_scopes, Wq, Wk, Wv, Wr, w_alpha, w_beta, Wo, bo,
           ln_g, ln_b):
    from concourse import bass_utils
    import ml_dtypes
    f8 = ml_dtypes.float8_e4m3

    edge_attr = np.asarray(edge_attr, dtype=np.float32)
    scopes = np.asarray(batch_scopes)
    Wq = np.asarray(Wq, np.float32); Wk = np.asarray(Wk, np.float32)
    Wv = np.asarray(Wv, np.float32); Wr = np.asarray(Wr, np.float32)
    Wo = np.asarray(Wo, np.float32)
    w_alpha = np.asarray(w_alpha, np.float32)
    w_beta = np.asarray(w_beta, np.float32)
    bo = np.asarray(bo, np.float32)
    ln_g = np.asarray(ln_g, np.float32); ln_b = np.asarray(ln_b, np.float32)

    assert np.all(scopes[:, 1] == L), "equal-length contiguous scopes expected"
    starts = scopes[:, 0].astype(np.int64)

    common, apply_bo, apply_affine = _host_consts(
        Wq, Wk, Wv, Wr, w_alpha, w_beta, Wo, bo, ln_g, ln_b)
    nc = _get_nc(apply_bo, apply_affine)

    in_maps = []
    for c in range(NCORES):
        rows = np.concatenate([
            np.arange(starts[c * G + g], starts[c * G + g] + L)
            for g in range(G)])
        xs = np.ascontiguousarray(edge_attr[rows].T) * XS     # [512, G*L]
        xhi = xs.astype(f8)
        xlo = (xs - xhi.astype(np.float32)).astype(f8)
        in_maps.append({"xhi": xhi, "xlo": xlo, **common})

    res = bass_utils.run_bass_kernel_spmd(nc, in_maps, core_ids=list(range(NCORES)))
    out = np.concatenate([r["out"] for r in res.results], axis=0)
    return out.astype(np.float32)


# revision 21
# speedup vs baseline: 1.0311x; 1.0167x over previous
"""BondFastAttention Trainium2 kernel (self-contained).

Shapes (hardcoded from the problem spec):
  edge_attr [65536, 512] fp32, B=64 graphs x L=1024 bonds, HID=512, 8 heads x D=64.
  8 NeuronCores, data-parallel over graphs: G=8 graphs per core.

Device layout: transposed domain - features on partitions, tokens on free dim.
Q/K/V projections run as 3-term error-compensated fp8e4 DoubleRow matmuls
(X,W split host-side into hi+lo fp8; the lo*lo term is dropped), giving
~2.25x fewer PE cycles than bf16 at ~0.1% error. The softmax-normalizer
broadcast (rbc) is one fp8 DoubleRow matmul per chunk using mean-centered
Delta-r against exact power-of-two selector weights. All activations are
computed in a x128-scaled basis (X*4, W*32); the final LayerNorm is
scale-invariant so the scaling washes out.
"""
import numpy as np

HID = 512
HEADS = 8
D = 64
B = 64
L = 1024
SCALE = D ** -0.5
EPS = 1e-5
NCORES = 8
G = B // NCORES          # graphs per core
NCH = HID // 128         # 4 feature chunks (2 heads each)
NT = L // 128            # 8 token chunks

XS = 4.0                 # X scale
WS = 32.0                # W scale  -> q' = 128 q
PS = XS * WS             # 128: projection output scale
R0 = 1.0 / 64
DRS = 2048.0             # Delta-r quantization scale (2^11)


def _build(apply_bo: bool, apply_affine: bool):
    import concourse.bass as bass
    from concourse import bacc
    import concourse.mybir as mybir
    from concourse.tile import TileContext

    F32 = mybir.dt.float32
    F8 = mybir.dt.float8e4
    BF16 = mybir.dt.bfloat16
    AT = mybir.ActivationFunctionType
    OP = mybir.AluOpType
    DR = mybir.MatmulPerfMode.DoubleRow

    nc = bacc.Bacc()

    # Restrict activation tables so Exp/Ln/Copy/Relu/Identity resolve to one
    # physical table load (natural_log_exp_and_others).
    import concourse.bacc as _bacc_mod
    _orig_gat = _bacc_mod.get_activation_tables

    def _gat(arch):
        t = _orig_gat(arch)
        ours = {AT.Exp, AT.Ln, AT.Copy, AT.Relu, AT.Identity}
        out = {}
        for k, funcs in t.items():
            if k == "natural_log_exp_and_others":
                out[k] = funcs
            else:
                out[k] = {f for f in funcs if f not in ours}
        return out

    xhi_d = nc.dram_tensor("xhi", [HID, G * L], F8, kind="ExternalInput")
    xlo_d = nc.dram_tensor("xlo", [HID, G * L], F8, kind="ExternalInput")
    # W hi/lo per projection: [128, c(2), k(2), 512] = W'[o, 256c+128k+p]
    w_d = {}
    for name in ("q", "k", "v"):
        w_d[name + "hi"] = nc.dram_tensor(f"w{name}hi", [128, 2 * 2 * HID], F8,
                                          kind="ExternalInput")
        w_d[name + "lo"] = nc.dram_tensor(f"w{name}lo", [128, 2 * 2 * HID], F8,
                                          kind="ExternalInput")
    wot_d = nc.dram_tensor("wot", [HID, HID], BF16, kind="ExternalInput")
    wrs_d = nc.dram_tensor("wrs", [128, 128], BF16, kind="ExternalInput")
    ident_d = nc.dram_tensor("ident", [128, 128], BF16, kind="ExternalInput")
    segs_d = nc.dram_tensor("segs", [128, 8 * NCH], BF16, kind="ExternalInput")
    # rbc-DR stationary: [8, k(2), 512]: k=0 sels*2^-5, k=1 sels
    rstat_d = nc.dram_tensor("rstat", [8, 2 * HID], F8, kind="ExternalInput")
    wa_d = nc.dram_tensor("wa", [128, 1], F32, kind="ExternalInput")
    wbs_d = nc.dram_tensor("wbs", [128, 1], F32, kind="ExternalInput")
    if apply_bo:
        bod = nc.dram_tensor("bo", [1, HID], F32, kind="ExternalInput")
        onesd = nc.dram_tensor("ones1", [1, 128], F32, kind="ExternalInput")
    if apply_affine:
        lngd = nc.dram_tensor("ln_g", [128, HID], F32, kind="ExternalInput")
        lnbd = nc.dram_tensor("ln_b", [128, HID], F32, kind="ExternalInput")
    outd = nc.dram_tensor("out", [G * L, HID], F32, kind="ExternalOutput")

    with TileContext(nc) as tc:
        with tc.tile_pool(name="consts", bufs=1) as cp, \
             tc.tile_pool(name="big", bufs=1) as bp, \
             tc.tile_pool(name="small", bufs=2) as sp, \
             tc.tile_pool(name="pq", bufs=3, space="PSUM") as pqp, \
             tc.tile_pool(name="ps", bufs=2, space="PSUM") as psp, \
             tc.tile_pool(name="pr", bufs=3, space="PSUM") as prp:

            def load_x(g):
                x_sb = {}
                for nm, dt_ in (("xhi", xhi_d), ("xlo", xlo_d)):
                    xt_all = bp.tile([128, NCH * L], F8, name=f"{nm}{g}", tag=nm,
                                     bufs=2)
                    src = bass.AP(
                        tensor=dt_.ap().tensor, offset=g * L,
                        ap=[[G * L, 128], [128 * G * L, NCH], [1, L]])
                    nc.sync.dma_start(
                        out=xt_all.rearrange("p (i l) -> p i l", i=NCH), in_=src)
                    x_sb[nm] = xt_all.rearrange("p (i l) -> p i l", i=NCH)
                return x_sb

            # ---- constants to SBUF (wq + x0 first so projQ(0) starts early) ----
            w_sb = {}
            for name in ("q",):
                for hl in ("hi", "lo"):
                    t = cp.tile([128, 2 * 2 * HID], F8, name=f"w{name}{hl}")
                    nc.sync.dma_start(out=t, in_=w_d[name + hl].ap())
                    w_sb[name + hl] = t.rearrange("p (c k o) -> p c k o", c=2, k=2)
            x_states = {0: load_x(0)}
            for name in ("k", "v"):
                for hl in ("hi", "lo"):
                    t = cp.tile([128, 2 * 2 * HID], F8, name=f"w{name}{hl}")
                    nc.sync.dma_start(out=t, in_=w_d[name + hl].ap())
                    w_sb[name + hl] = t.rearrange("p (c k o) -> p c k o", c=2, k=2)
            wo_sb = [cp.tile([128, HID], BF16, name=f"wo{i}") for i in range(NCH)]
            for i in range(NCH):
                nc.sync.dma_start(out=wo_sb[i], in_=wot_d.ap()[128 * i:128 * (i + 1), :])
            wrs_sb = cp.tile([128, 128], BF16, name="wrs")
            nc.sync.dma_start(out=wrs_sb, in_=wrs_d.ap())
            ident_sb = cp.tile([128, 128], BF16, name="ident")
            nc.sync.dma_start(out=ident_sb, in_=ident_d.ap())
            segs_sb = cp.tile([128, 8 * NCH], BF16, name="segs")
            nc.sync.dma_start(out=segs_sb, in_=segs_d.ap())
            rstat_sb_ = cp.tile([8, 2 * HID], F8, name="rstat")
            nc.sync.dma_start(out=rstat_sb_, in_=rstat_d.ap())
            rstat_sb = rstat_sb_.rearrange("p (k o) -> p k o", k=2)
            wa_sb = cp.tile([128, 1], F32, name="wa")
            nc.sync.dma_start(out=wa_sb, in_=wa_d.ap())
            wbs_sb = cp.tile([128, 1], F32, name="wbs")
            nc.sync.dma_start(out=wbs_sb, in_=wbs_d.ap())
            # rbc-DR moving tiles, double-buffered by graph parity:
            # [8, k(2), 1024]: k=0 Delta-r8 (written per graph), k=1 ones.
            rmov = {}
            for st in ("A", "B"):
                for par in (0, 1):
                    t = cp.tile([8, 2 * L], F8, name=f"rmov{st}{par}")
                    nc.vector.memset(t[:, L:2 * L], 1.0)
                    rmov[(st, par)] = t
            if apply_bo:
                ones1_sb = cp.tile([1, 128], F32, name="ones1")
                nc.sync.dma_start(out=ones1_sb, in_=onesd.ap().bitcast(F32))
                bo_sb = cp.tile([1, HID], F32, name="bo")
                nc.sync.dma_start(out=bo_sb, in_=bod.ap())
            if apply_affine:
                lng_sb = cp.tile([128, HID], F32, name="lng")
                nc.sync.dma_start(out=lng_sb, in_=lngd.ap())
                lnb_sb = cp.tile([128, HID], F32, name="lnb")
                nc.sync.dma_start(out=lnb_sb, in_=lnbd.ap())

            def proj3h(name, j, n0, xhi, xlo, pp):
                """Compensated fp8 DR proj: out-chunk j, token half n0.
                K uses the plain hi@hi term only (its error is attenuated
                through the near-cancelling beta path); Q/V use 3 terms."""
                whi = w_sb[name + "hi"]
                wlo = w_sb[name + "lo"] if name != "k" else None
                o0, o1 = 128 * j, 128 * (j + 1)
                first = True
                for c in (0, 1):
                    mhi = xhi[:, 2 * c:2 * c + 2, n0:n0 + 512]
                    mlo = xlo[:, 2 * c:2 * c + 2, n0:n0 + 512]
                    whis = whi[:, c, :, o0:o1]
                    if wlo is None:
                        terms = ((whis, mhi),)
                    else:
                        wlos = wlo[:, c, :, o0:o1]
                        terms = ((whis, mhi), (whis, mlo), (wlos, mhi))
                    for ti, (wsl, msl) in enumerate(terms):
                        nc.tensor.matmul(
                            pp, wsl, msl, start=first,
                            stop=(c == 1 and ti == len(terms) - 1),
                            perf_mode=DR)
                        first = False

            def wo_ln_setup(g):
                return {
                    "mv": sp.tile([128, 2 * NT], F32, name=f"mv{g}", tag="mv"),
                    "vf": sp.tile([128, NT], F32, name=f"vf{g}", tag="vf"),
                    "lnv": sp.tile([128, NT], F32, name=f"lnv{g}", tag="lnv"),
                    "rstd": sp.tile([128, NT], F32, name=f"rstd{g}", tag="rstd"),
                    "nmr": sp.tile([128, NT], F32, name=f"nmr{g}", tag="nmr"),
                }

            def emit_wo_t(g, att_all, ws, t):
                """Wo GEMM + LayerNorm for one token chunk of graph g.
                PSUM frees via a single Act copy; LN chain runs on SBUF."""
                mv_all = ws["mv"]
                o_ps = psp.tile([128, 512], F32, name=f"ops{g}{t}", tag="seg")
                for j in range(NCH):
                    nc.tensor.matmul(
                        o_ps, att_all[:, j * L + 128 * t:j * L + 128 * (t + 1)],
                        wo_sb[j], start=(j == 0),
                        stop=(j == NCH - 1 and not apply_bo))
                if apply_bo:
                    nc.tensor.matmul(o_ps, ones1_sb, bo_sb, start=False,
                                     stop=True)
                osb = sp.tile([128, HID], F32, name=f"osb{g}{t}", tag="osb",
                              bufs=4)
                nc.scalar.copy(out=osb, in_=o_ps)
                stats = sp.tile([128, 6], F32, name=f"st{g}{t}", tag="st",
                                bufs=3)
                nc.vector.bn_stats(out=stats, in_=osb)
                nc.vector.bn_aggr(out=mv_all[:, 2 * t:2 * t + 2], in_=stats)
                nc.scalar.activation(out=ws["vf"][:, t:t + 1],
                                     in_=mv_all[:, 2 * t + 1:2 * t + 2],
                                     func=AT.Copy, bias=EPS)
                nc.scalar.activation(out=ws["lnv"][:, t:t + 1],
                                     in_=ws["vf"][:, t:t + 1], func=AT.Ln)
                nc.scalar.activation(out=ws["rstd"][:, t:t + 1],
                                     in_=ws["lnv"][:, t:t + 1],
                                     func=AT.Exp, scale=-0.5)
                # out = (osb - mu) * rstd, in place on SBUF
                nc.vector.tensor_scalar(
                    out=osb, in0=osb, scalar1=mv_all[:, 2 * t:2 * t + 1],
                    scalar2=ws["rstd"][:, t:t + 1],
                    op0=OP.subtract, op1=OP.mult)
                if apply_affine:
                    nc.vector.tensor_mul(out=osb, in0=osb, in1=lng_sb)
                    nc.vector.tensor_add(out=osb, in0=osb, in1=lnb_sb)
                nc.gpsimd.dma_start(
                    out=outd.ap()[g * L + 128 * t:g * L + 128 * (t + 1), :],
                    in_=osb)

            prev_att = None
            for g in range(G):
                par = g % 2
                x_sb = x_states.pop(g)
                if g + 1 < G:
                    x_states[g + 1] = load_x(g + 1)   # full-iteration prefetch

                # ---- Q projections: PE proj, Act e, DVE qsb copy, Pool m;
                #      segA_j trails one chunk behind to avoid PE stalls ----
                qsb_all = bp.tile([128, NCH * L], BF16, name=f"qsb{g}", tag="qsb",
                                  bufs=2)
                e_all = bp.tile([128, NCH * L], BF16, name=f"e{g}", tag="e", bufs=2)
                ksb_all = bp.tile([128, NCH * L], BF16, name=f"ksb{g}", tag="ksb",
                                  bufs=2)
                sA_lo = psp.tile([8, 512], F32, name=f"sAlo{g}", tag="seg",
                                 padded_shape=[128, 512])
                sA_hi = psp.tile([8, 512], F32, name=f"sAhi{g}", tag="seg",
                                 padded_shape=[128, 512])
                m_all = []

                def seg_emit(j, src_all, lo, hi_):
                    ech = src_all[:, j * L:(j + 1) * L]
                    for n0, s_ps in ((0, lo), (512, hi_)):
                        nc.tensor.matmul(
                            s_ps, segs_sb[:, 8 * j:8 * (j + 1)],
                            ech[:, n0:n0 + 512],
                            start=(j == 0), stop=(j == NCH - 1))

                for j in range(NCH):
                    for n0 in (0, 512):
                        pp = pqp.tile([128, 512], F32, name=f"ppq{g}{j}{n0}",
                                      tag="pq")
                        proj3h("q", j, n0, x_sb["xhi"], x_sb["xlo"], pp)
                        nc.scalar.activation(
                            out=e_all[:, j * L + n0:j * L + n0 + 512], in_=pp,
                            func=AT.Exp, scale=wa_sb)
                        nc.vector.tensor_copy(
                            out=qsb_all[:, j * L + n0:j * L + n0 + 512], in_=pp)
                    ech = e_all[:, j * L:(j + 1) * L]
                    mj = sp.tile([128, L], BF16, name=f"m{g}{j}", tag="m", bufs=5)
                    nc.gpsimd.tensor_tensor(
                        out=mj, in0=ech, in1=qsb_all[:, j * L:(j + 1) * L],
                        op=OP.mult)
                    m_all.append(mj)
                    if j >= 1:
                        seg_emit(j - 1, e_all, sA_lo, sA_hi)

                # ---- K projections (copy only; eb later from SBUF) ----
                for j in range(NCH):
                    for n0 in (0, 512):
                        pp = pqp.tile([128, 512], F32, name=f"ppk{g}{j}{n0}",
                                      tag="pq")
                        proj3h("k", j, n0, x_sb["xhi"], x_sb["xlo"], pp)
                        nc.scalar.copy(
                            out=ksb_all[:, j * L + n0:j * L + n0 + 512], in_=pp)
                    if j == 0:
                        seg_emit(NCH - 1, e_all, sA_lo, sA_hi)

                # ---- recip A + Delta-r ----
                rmovA = rmov[("A", par)]
                rtA = sp.tile([8, L], F32, name=f"rtA{g}", tag="rt")
                nc.vector.reciprocal_approx_fast(out=rtA[:, 0:512], in_=sA_lo)
                nc.vector.reciprocal_approx_fast(out=rtA[:, 512:L], in_=sA_hi)
                nc.vector.tensor_scalar(
                    out=rmovA[:, 0:L], in0=rtA, scalar1=R0, scalar2=DRS,
                    op0=OP.subtract, op1=OP.mult)

                # ---- per-j: rbcA + STT-A -> gq_j -> gqwb_j -> eb_j -> m_b_j
                #      -> segB_j, with projV_j as PE filler ----
                vsb_all = bp.tile([128, NCH * L], BF16, name=f"vsb{g}", tag="vsb",
                                  bufs=2)
                eb_all = bp.tile([128, NCH * L], BF16, name=f"eb{g}", tag="e",
                                 bufs=2)
                sB_lo = psp.tile([8, 512], F32, name=f"sBlo{g}", tag="seg",
                                 padded_shape=[128, 512])
                sB_hi = psp.tile([8, 512], F32, name=f"sBhi{g}", tag="seg",
                                 padded_shape=[128, 512])
                gqh = sp.tile([128, 2 * NCH], F32, name=f"gqh{g}", tag="gqh")
                gq = sp.tile([128, NCH], F32, name=f"gq{g}", tag="gq")
                gqwb = sp.tile([128, NCH], F32, name=f"gqwb{g}", tag="gqwb")
                scr = sp.tile([128, 512], BF16, name=f"scrA{g}", tag="scr", bufs=3)
                mb_all = []
                for j in range(NCH):
                    for hi, n0 in enumerate((0, 512)):
                        rbc = prp.tile([128, 512], F32, name=f"rbA{g}{j}{n0}",
                                       tag="pr")
                        nc.tensor.matmul(
                            rbc, rstat_sb[:, :, 128 * j:128 * (j + 1)],
                            rmovA.rearrange("p (k l) -> p k l", k=2)[:, :, n0:n0 + 512],
                            perf_mode=DR)
                        nc.vector.scalar_tensor_tensor(
                            out=scr, in0=m_all[j][:, n0:n0 + 512], scalar=R0,
                            in1=rbc, op0=OP.mult, op1=OP.mult,
                            accum_out=gqh[:, 2 * j + hi:2 * j + hi + 1])
                    nc.vector.tensor_add(out=gq[:, j:j + 1],
                                         in0=gqh[:, 2 * j:2 * j + 1],
                                         in1=gqh[:, 2 * j + 1:2 * j + 2])
                    nc.vector.tensor_scalar_mul(out=gqwb[:, j:j + 1],
                                                in0=gq[:, j:j + 1], scalar1=wbs_sb)
                    ebch = eb_all[:, j * L:(j + 1) * L]
                    nc.scalar.activation(out=ebch,
                                         in_=ksb_all[:, j * L:(j + 1) * L],
                                         func=AT.Exp, scale=gqwb[:, j:j + 1])
                    mbj = sp.tile([128, L], BF16, name=f"mb{g}{j}", tag="m", bufs=5)
                    nc.gpsimd.tensor_tensor(out=mbj, in0=ebch,
                                            in1=ksb_all[:, j * L:(j + 1) * L],
                                            op=OP.mult)
                    mb_all.append(mbj)
                    # PE filler while the eb chain runs
                    for n0 in (0, 512):
                        pp = pqp.tile([128, 512], F32, name=f"ppv{g}{j}{n0}",
                                      tag="pq")
                        proj3h("v", j, n0, x_sb["xhi"], x_sb["xlo"], pp)
                        nc.scalar.copy(
                            out=vsb_all[:, j * L + n0:j * L + n0 + 512], in_=pp)
                    seg_emit(j, eb_all, sB_lo, sB_hi)

                # ---- recip B + Delta-r ----
                rmovB = rmov[("B", par)]
                rtB = sp.tile([8, L], F32, name=f"rtB{g}", tag="rt")
                nc.vector.reciprocal_approx_fast(out=rtB[:, 0:512], in_=sB_lo)
                nc.vector.reciprocal_approx_fast(out=rtB[:, 512:L], in_=sB_hi)
                nc.vector.tensor_scalar(
                    out=rmovB[:, 0:L], in0=rtB, scalar1=R0, scalar2=DRS,
                    op0=OP.subtract, op1=OP.mult)

                # ---- per-j: rbcB + STT-B -> gk_j -> kv_j (DVE 4x) -> kvout_j
                #      -> att_j, with Wo/LN(g-1) token-pairs as PE filler ----
                att_all = bp.tile([128, NCH * L], BF16, name=f"att{g}", tag="att",
                                  bufs=2)
                acch = sp.tile([128, 2 * NCH], F32, name=f"acch{g}", tag="gqh")
                gk = sp.tile([128, NCH], F32, name=f"gk{g}", tag="gk")
                scrb = sp.tile([128, 512], BF16, name=f"scrB{g}", tag="scr", bufs=3)
                if prev_att is not None:
                    ws_prev = wo_ln_setup(g - 1)
                for j in range(NCH):
                    for hi, n0 in enumerate((0, 512)):
                        rbc = prp.tile([128, 512], F32, name=f"rbB{g}{j}{n0}",
                                       tag="pr")
                        nc.tensor.matmul(
                            rbc, rstat_sb[:, :, 128 * j:128 * (j + 1)],
                            rmovB.rearrange("p (k l) -> p k l", k=2)[:, :, n0:n0 + 512],
                            perf_mode=DR)
                        nc.vector.scalar_tensor_tensor(
                            out=scrb, in0=mb_all[j][:, n0:n0 + 512], scalar=R0,
                            in1=rbc, op0=OP.mult, op1=OP.mult,
                            accum_out=acch[:, 2 * j + hi:2 * j + hi + 1])
                    nc.vector.tensor_add(out=acch[:, 2 * j:2 * j + 1],
                                         in0=acch[:, 2 * j:2 * j + 1],
                                         in1=acch[:, 2 * j + 1:2 * j + 2])
                    nc.vector.tensor_mul(out=gk[:, j:j + 1],
                                         in0=acch[:, 2 * j:2 * j + 1],
                                         in1=gq[:, j:j + 1])
                    kvj = sp.tile([128, L], BF16, name=f"kv{g}{j}", tag="kv",
                                  bufs=3)
                    nc.vector.tensor_scalar_mul(out=kvj,
                                                in0=vsb_all[:, j * L:(j + 1) * L],
                                                scalar1=gk[:, j:j + 1])
                    for n0 in (0, 512):
                        kvo = prp.tile([128, 512], F32, name=f"kvo{g}{j}{n0}",
                                       tag="pr")
                        nc.tensor.matmul(kvo, ident_sb,
                                         qsb_all[:, j * L + n0:j * L + n0 + 512],
                                         start=True, stop=False)
                        nc.tensor.matmul(kvo, wrs_sb, kvj[:, n0:n0 + 512],
                                         start=False, stop=True)
                        nc.scalar.activation(
                            out=att_all[:, j * L + n0:j * L + n0 + 512],
                            in_=kvo, func=AT.Relu)
                    if prev_att is not None:
                        emit_wo_t(g - 1, prev_att, ws_prev, 2 * j)
                        emit_wo_t(g - 1, prev_att, ws_prev, 2 * j + 1)
                prev_att = att_all

            ws_last = wo_ln_setup(G - 1)
            for t in range(NT):
                emit_wo_t(G - 1, prev_att, ws_last, t)

    _bacc_mod.get_activation_tables = _gat
    try:
        nc.compile()
    finally:
        _bacc_mod.get_activation_tables = _orig_gat
    return nc


_NC_CACHE = {}


def _get_nc(apply_bo=False, apply_affine=False):
    key = (apply_bo, apply_affine)
    if key not in _NC_CACHE:
        _NC_CACHE[key] = _build(apply_bo, apply_affine)
    return _NC_CACHE[key]


def _host_consts(Wq, Wk, Wv, Wr, w_alpha, w_beta, Wo, bo, ln_g, ln_b):
    import ml_dtypes
    f8 = ml_dtypes.float8_e4m3
    bf = ml_dtypes.bfloat16

    def whl(W):
        # [128, c, k, o] with value W'[o, 256c+128k+p] * WS, split hi/lo fp8
        Ws = (W.T * WS).astype(np.float32)          # [h, o]
        t = Ws.reshape(2, 2, 128, HID).transpose(2, 0, 1, 3)   # [p, c, k, o]
        hi = t.astype(f8)
        lo = (t - hi.astype(np.float32)).astype(f8)
        return (np.ascontiguousarray(hi.reshape(128, 2 * 2 * HID)),
                np.ascontiguousarray(lo.reshape(128, 2 * 2 * HID)))

    qhi, qlo = whl(Wq)
    khi, klo = whl(Wk)
    vhi, vlo = whl(Wv)
    wot = np.ascontiguousarray(Wo.T).astype(bf)
    # wrs = block-diag(Wr.T) * PS / (PS^2 * gk-scale) ... in the scaled basis:
    # kv' = 2^21 kv_true, want kvout' = PS*kvout_true -> wrs = wrbd * 2^-14
    wrt = Wr.T.astype(np.float32)
    wrbd = np.zeros((128, 128), np.float32)
    wrbd[:64, :64] = wrt
    wrbd[64:, 64:] = wrt
    wrs = (wrbd * (1.0 / (PS * PS))).astype(bf)
    ident = np.eye(128, dtype=np.float32).astype(bf)
    segs = np.zeros((128, 8 * NCH), np.float32)
    for j in range(NCH):
        for p in range(128):
            segs[p, 8 * j + 2 * j + p // 64] = 1.0
    # rstat [8, k(2), 512]: k=0 sels * 2^(6-11) = 2^-5, k=1 sels (exact 1.0)
    sels = np.zeros((8, HID), np.float32)
    for m in range(HID):
        sels[2 * (m // 128) + (m % 128) // 64, m] = 1.0
    rstat = np.stack([sels * (64.0 / DRS), sels], axis=1).reshape(8, 2 * HID)
    wa_col = (np.tile(w_alpha, 2) * SCALE / PS).reshape(128, 1).astype(np.float32)
    wbs_col = (np.tile(w_beta, 2) * SCALE / (PS * PS)).reshape(128, 1).astype(
        np.float32)

    common = {"wqhi": qhi, "wqlo": qlo, "wkhi": khi, "wklo": klo,
              "wvhi": vhi, "wvlo": vlo, "wot": wot, "wrs": wrs,
              "ident": ident, "segs": segs.astype(bf),
              "rstat": rstat.astype(f8), "wa": wa_col, "wbs": wbs_col}
    apply_bo = not np.allclose(bo, 0.0)
    apply_affine = not (np.allclose(ln_g, 1.0) and np.allclose(ln_b, 0.0))
    if apply_bo:
        common["bo"] = (bo * PS).reshape(1, HID).astype(np.float32)
        common["ones1"] = np.ones((1, 128), np.float32)
    if apply_affine:
        common["ln_g"] = np.tile(ln_g, (128, 1)).astype(np.float32)
        common["ln_b"] = np.tile(ln_b, (128, 1)).astype(np.float32)
    return common, apply_bo, apply_affine


def kernel(edge_attr, batch_scopes, Wq, Wk, Wv, Wr, w_alpha, w_beta, Wo, bo,
           ln_g, ln_b):
    from concourse import bass_utils
    import ml_dtypes
    f8 = ml_dtypes.float8_e4m3

    edge_attr = np.asarray(edge_attr, dtype=np.float32)
    scopes = np.asarray(batch_scopes)
    Wq = np.asarray(Wq, np.float32); Wk = np.asarray(Wk, np.float32)
    Wv = np.asarray(Wv, np.float32); Wr = np.asarray(Wr, np.float32)
    Wo = np.asarray(Wo, np.float32)
    w_alpha = np.asarray(w_alpha, np.float32)
    w_beta = np.asarray(w_beta, np.float32)
    bo = np.asarray(bo, np.float32)
    ln_g = np.asarray(ln_g, np.float32); ln_b = np.asarray(ln_b, np.float32)

    assert np.all(scopes[:, 1] == L), "equal-length contiguous scopes expected"
    starts = scopes[:, 0].astype(np.int64)

    common, apply_bo, apply_affine = _host_consts(
        Wq, Wk, Wv, Wr, w_alpha, w_beta, Wo, bo, ln_g, ln_b)
    nc = _get_nc(apply_bo, apply_affine)

    in_maps = []
    for c in range(NCORES):
        rows = np.concatenate([
            np.arange(starts[c * G + g], starts[c * G + g] + L)
            for g in range(G)])
        xs = np.ascontiguousarray(edge_attr[rows].T) * XS     # [512, G*L]
        xhi = xs.astype(f8)
        xlo = (xs - xhi.astype(np.float32)).astype(f8)
        in_maps.append({"xhi": xhi, "xlo": xlo, **common})

    res = bass_utils.run_bass_kernel_spmd(nc, in_maps, core_ids=list(range(NCORES)))
    out = np.concatenate([r["out"] for r in res.results], axis=0)
    return out.astype(np.float32)
